# revision 9
# baseline (speedup 1.0000x reference)
"""DGCNN forward on 8 TRN2 NeuronCores — pure data parallel over batch.

Layout: feature-major xT[C, N] (channels in partitions, points in free dim).
EdgeConv:  E = max(v, 0.2*v),  v = gs*(max_k A[idx[n,k]] + Bc[n]) + b
with A = x@WnT, Bc = x@(Wc-Wn)T precomputed per layer (max over neighbors
commutes with the monotone per-channel affine + LeakyReLU since gs > 0).

kNN top-20: rank rows of s'[n,j] = <x_n,x_j> - xx_j/2 (same order per row as
the reference's pairwise metric), via DVE max/max_index/match_replace
(3 rounds of 8 -> top-24 values+indices, take first 20 = exact top-20 set).

Execution: a persistent _Runner holds the jitted PJRT shard_map callable
and keeps all weight operands device-resident across kernel() calls
(re-uploaded only if their bytes change), so a steady-state call ships just
xT (24 KB/core) + output zeros and pays a single execute+fetch round trip
over the axon tunnel. End-to-end per-call wall time is dominated by that
round trip (~40 ms); on-device execution is ~ms-scale.

Gather: per (tile, 128-channel half) three chunked dma_gathers (<=1024
descriptors each, the SWDGE ring limit) pulling rows of a DRAM A-table;
indices are folded into the wrapped-16 int16 layout the instruction expects
via a handful of strided DMAs per pair of tiles. Emission is software-
pipelined (scores two tiles ahead, gather jobs consumed one pair behind)
so the in-order DVE queue never stalls on gather latency.
"""
import numpy as np

import concourse.bass as bass
import concourse.mybir as mybir
import concourse.tile as tile
from concourse import bacc
from concourse.masks import make_identity

F32 = mybir.dt.float32
U16 = mybir.dt.uint16
U32 = mybir.dt.uint32
I16 = mybir.dt.int16

KNN = 20
NCORES = 8
CS = [3, 64, 64, 128, 256]     # conv input channels
COS = [64, 64, 128, 256]       # conv output channels
EPS = 1e-5
BN_SCALE = 1.0 / np.sqrt(1.0 + EPS)
NEG = -3.0e38


def _blocks(c):
    out = []
    off = 0
    while off < c:
        b = min(128, c - off)
        out.append((off, b))
        off += b
    return out


def build(N=2048, debug=False):
    """Emit the per-core Bass program. Returns compiled nc."""
    nc = bacc.Bacc("TRN2", target_bir_lowering=False, debug=debug,
                   num_swdge_queues=4)
    NT = N // 128
    CHW = min(N, 512)
    NCH = N // CHW

    # ---------------- DRAM I/O ----------------
    xT_d = nc.dram_tensor("xT", [CS[0], N], F32, kind="ExternalInput")
    wn_d, wd_d, gs_d, bb_d = [], [], [], []
    for l in range(4):
        wn_d.append(nc.dram_tensor(f"WnT{l}", [CS[l], COS[l]], F32, kind="ExternalInput"))
        wd_d.append(nc.dram_tensor(f"WdT{l}", [CS[l], COS[l]], F32, kind="ExternalInput"))
        gs_d.append(nc.dram_tensor(f"gs{l}", [COS[l], 1], F32, kind="ExternalInput"))
        bb_d.append(nc.dram_tensor(f"bb{l}", [COS[l], 1], F32, kind="ExternalInput"))
    w5_d = nc.dram_tensor("W5T", [512, 1024], F32, kind="ExternalInput")
    gs5_d = nc.dram_tensor("gs5", [1024, 1], F32, kind="ExternalInput")
    bb5_d = nc.dram_tensor("bb5", [1024, 1], F32, kind="ExternalInput")
    l1_d = nc.dram_tensor("L1T", [2048, 512], F32, kind="ExternalInput")
    gs6_d = nc.dram_tensor("gs6r", [1, 512], F32, kind="ExternalInput")
    bb6_d = nc.dram_tensor("bb6r", [1, 512], F32, kind="ExternalInput")
    l2_d = nc.dram_tensor("L2T", [512, 256], F32, kind="ExternalInput")
    gs7_d = nc.dram_tensor("gs7r", [1, 256], F32, kind="ExternalInput")
    bb7_d = nc.dram_tensor("bb7r", [1, 256], F32, kind="ExternalInput")
    l3_d = nc.dram_tensor("L3T", [256, 10], F32, kind="ExternalInput")
    l3b_d = nc.dram_tensor("L3br", [1, 10], F32, kind="ExternalInput")
    out_d = nc.dram_tensor("out", [1, 10], F32, kind="ExternalOutput")

    with tile.TileContext(nc) as tc:
        import contextlib
        ctx = contextlib.ExitStack()
        with ctx:
            cpool = ctx.enter_context(tc.tile_pool(name="const", bufs=1))
            xpool = ctx.enter_context(tc.tile_pool(name="xt", bufs=1))
            spool = ctx.enter_context(tc.tile_pool(name="ssb", bufs=3))
            gpool = ctx.enter_context(tc.tile_pool(name="gat", bufs=3))
            wpool = ctx.enter_context(tc.tile_pool(name="work", bufs=2))
            strm = ctx.enter_context(tc.tile_pool(name="strm", bufs=2))
            dram = ctx.enter_context(tc.tile_pool(name="dram", bufs=2, space="DRAM"))
            s_ps = ctx.enter_context(tc.tile_pool(name="s_ps", bufs=3, space="PSUM"))
            t_ps = ctx.enter_context(tc.tile_pool(name="t_ps", bufs=1, space="PSUM"))
            r_ps = ctx.enter_context(tc.tile_pool(name="r_ps", bufs=3, space="PSUM"))

            # ---------------- constants / weights to SBUF ----------------
            ident = cpool.tile([128, 128], F32, tag="ident")
            make_identity(nc, ident[:])
            ones128 = cpool.tile([1, 128], F32, tag="ones128")
            nc.vector.memset(ones128[:], 1.0)
            neghalf = cpool.tile([128, 1], F32, tag="neghalf")
            nc.vector.memset(neghalf[:], -0.5)

            wn_sb, wd_sb, gs_sb, bb_sb = [], [], [], []
            for l in range(4):
                wn_b, wd_b, gs_b, bb_b = [], [], [], []
                for (off, b) in _blocks(CS[l]):
                    t1 = cpool.tile([b, COS[l]], F32, tag=f"wn{l}_{off}")
                    nc.sync.dma_start(t1[:], wn_d[l][off:off + b, :])
                    wn_b.append(t1)
                    t2 = cpool.tile([b, COS[l]], F32, tag=f"wd{l}_{off}")
                    nc.sync.dma_start(t2[:], wd_d[l][off:off + b, :])
                    wd_b.append(t2)
                for (off, b) in _blocks(COS[l]):
                    t3 = cpool.tile([b, 1], F32, tag=f"gs{l}_{off}")
                    nc.sync.dma_start(t3[:], gs_d[l][off:off + b, :])
                    gs_b.append(t3)
                    t4 = cpool.tile([b, 1], F32, tag=f"bb{l}_{off}")
                    nc.sync.dma_start(t4[:], bb_d[l][off:off + b, :])
                    bb_b.append(t4)
                wn_sb.append(wn_b); wd_sb.append(wd_b)
                gs_sb.append(gs_b); bb_sb.append(bb_b)

            w5_rows = [(0, 64), (64, 64), (128, 128), (256, 128), (384, 128)]
            gs5_sb, bb5_sb = [], []
            for (off, b) in _blocks(1024):
                t = cpool.tile([b, 1], F32, tag=f"gs5_{off}")
                nc.sync.dma_start(t[:], gs5_d[off:off + b, :]); gs5_sb.append(t)
                t = cpool.tile([b, 1], F32, tag=f"bb5_{off}")
                nc.sync.dma_start(t[:], bb5_d[off:off + b, :]); bb5_sb.append(t)
            l2_sb = []
            for (off, b) in _blocks(512):
                t = cpool.tile([b, 256], F32, tag=f"l2_{off}")
                nc.sync.dma_start(t[:], l2_d[off:off + b, :]); l2_sb.append(t)
            l3_sb = []
            for (off, b) in _blocks(256):
                t = cpool.tile([b, 10], F32, tag=f"l3_{off}")
                nc.sync.dma_start(t[:], l3_d[off:off + b, :]); l3_sb.append(t)
            gs6r = cpool.tile([1, 512], F32, tag="gs6r")
            nc.sync.dma_start(gs6r[:], gs6_d[:])
            bb6r = cpool.tile([1, 512], F32, tag="bb6r")
            nc.sync.dma_start(bb6r[:], bb6_d[:])
            gs7r = cpool.tile([1, 256], F32, tag="gs7r")
            nc.sync.dma_start(gs7r[:], gs7_d[:])
            bb7r = cpool.tile([1, 256], F32, tag="bb7r")
            nc.sync.dma_start(bb7r[:], bb7_d[:])
            l3br = cpool.tile([1, 10], F32, tag="l3br")
            nc.sync.dma_start(l3br[:], l3b_d[:])

            xT0 = xpool.tile([CS[0], N], F32, tag="xT0")
            nc.sync.dma_start(xT0[:], xT_d[:])

            xT_blocks = [xT0]
            saved_xT = []

            # =================== EdgeConv layers ===================
            for l in range(4):
                C, Co = CS[l], COS[l]
                cblks = _blocks(C)
                oblks = _blocks(Co)

                # ---- sqm[j] = -xx_j/2 ----
                xsq = []
                for bi, (off, b) in enumerate(cblks):
                    t = spool.tile([128, N], F32, tag="s_sb")
                    nc.scalar.square(t[:b, :], xT_blocks[bi][:])
                    xsq.append(t)
                sqm = cpool.tile([1, N], F32, tag="sqm")
                for ch in range(NCH):
                    sl = bass.ts(ch, CHW)
                    ps = t_ps.tile([1, CHW], F32, tag="vec_ps")
                    for bi, (off, b) in enumerate(cblks):
                        nc.tensor.matmul(ps[:], lhsT=neghalf[:b, :],
                                         rhs=xsq[bi][:b, sl],
                                         start=(bi == 0), stop=(bi == len(cblks) - 1))
                    nc.scalar.copy(sqm[:, sl], ps[:])

                # ---- A table to DRAM + Bc table in SBUF (point-major);
                # emission deferred until after the first two score tiles ----
                A_dram = dram.tile([N, Co], F32, tag="Adram")
                Bc_all = wpool.tile([128, NT * Co], F32, tag="Bc_all",
                                    name="Bc_all", bufs=1)

                def emit_tables(t):
                    tsl = bass.ts(t, 128)
                    ps = t_ps.tile([128, Co], F32, tag="A_ps", name="A_ps")
                    for bi, (off, b) in enumerate(cblks):
                        nc.tensor.matmul(ps[:], lhsT=xT_blocks[bi][:, tsl],
                                         rhs=wn_sb[l][bi][:],
                                         start=(bi == 0), stop=(bi == len(cblks) - 1))
                    asb = wpool.tile([128, Co], F32, tag="A_sb", name="A_sb")
                    nc.scalar.copy(asb[:], ps[:])
                    nc.sync.dma_start(A_dram[t * 128:(t + 1) * 128, :], asb[:])
                    ps2 = t_ps.tile([128, Co], F32, tag="A_ps", name="ps2")
                    for bi, (off, b) in enumerate(cblks):
                        nc.tensor.matmul(ps2[:], lhsT=xT_blocks[bi][:, tsl],
                                         rhs=wd_sb[l][bi][:],
                                         start=(bi == 0), stop=(bi == len(cblks) - 1))
                    nc.scalar.copy(Bc_all[:, t * Co:(t + 1) * Co], ps2[:])

                nxt = [xpool.tile([b, N], F32, tag=f"xT{l + 1}_{off}",
                                  name=f"xT{l + 1}_{off}")
                       for (off, b) in oblks]

                # per-conv index buffers: raw [p, t*20+r] and wrapped-16
                big_idx = wpool.tile([128, NT * KNN], U16, tag="big_idx",
                                     name="big_idx", bufs=1)
                i16f = wpool.tile([128, NT * 160], U16, tag="i16f", name="i16f",
                                  bufs=1)

                ohalves = [(off, min(128, Co - off)) for off in range(0, Co, 128)]
                H = len(ohalves)

                if l == 3:
                    bands = [saved_xT[0][0], saved_xT[1][0], saved_xT[2][0],
                             nxt[0], nxt[1]]
                    hm_acc = [wpool.tile([128, 1], F32, tag=f"hm{m}",
                                         name=f"hm{m}", bufs=1) for m in range(8)]
                    sv_acc = [wpool.tile([128, 1], F32, tag=f"svA{m}",
                                         name=f"svA{m}", bufs=1) for m in range(8)]
                    sa_acc = [wpool.tile([128, 1], F32, tag=f"saA{m}",
                                         name=f"saA{m}", bufs=1) for m in range(8)]

                def emit_h_chunk(ch):
                    # one N-chunk of h = lrelu-affine(xc @ W5T): needs only
                    # conv4 output columns [ch*CHW, (ch+1)*CHW)
                    sl = bass.ts(ch, CHW)
                    for mb in range(8):
                        w5_sb = []
                        for ki, (off, b) in enumerate(w5_rows):
                            wt = strm.tile([b, 128], F32, tag=f"w5s_{ki}",
                                           name=f"w5s_{ki}")
                            nc.sync.dma_start(wt[:], w5_d[off:off + b,
                                                          mb * 128:(mb + 1) * 128])
                            w5_sb.append(wt)
                        ps = s_ps.tile([128, CHW], F32, tag="s_ps", name="h_ps")
                        for k in range(5):
                            nc.tensor.matmul(ps[:], lhsT=w5_sb[k][:],
                                             rhs=bands[k][:, sl],
                                             start=(k == 0), stop=(k == 4))
                        v5 = strm.tile([128, CHW], F32, tag="v5", name="v5")
                        svp = wpool.tile([128, 1], F32, tag="svp", name="svp")
                        nc.scalar.activation(v5[:], ps[:],
                                             mybir.ActivationFunctionType.Identity,
                                             bias=bb5_sb[mb][:], scale=gs5_sb[mb][:],
                                             accum_out=svp[:])
                        a5 = strm.tile([128, CHW], F32, tag="a5", name="a5")
                        sap = wpool.tile([128, 1], F32, tag="sap", name="sap")
                        nc.scalar.activation(a5[:], ps[:],
                                             mybir.ActivationFunctionType.Abs,
                                             bias=bb5_sb[mb][:], scale=gs5_sb[mb][:],
                                             accum_out=sap[:])
                        rmp = wpool.tile([128, 1], F32, tag="rmp", name="rmp")
                        nc.vector.tensor_reduce(out=rmp[:], in_=v5[:],
                                                axis=mybir.AxisListType.X,
                                                op=mybir.AluOpType.max)
                        if ch == 0:
                            nc.vector.tensor_copy(hm_acc[mb][:], rmp[:])
                            nc.vector.tensor_copy(sv_acc[mb][:], svp[:])
                            nc.vector.tensor_copy(sa_acc[mb][:], sap[:])
                        else:
                            nc.vector.tensor_tensor(hm_acc[mb][:], hm_acc[mb][:],
                                                    rmp[:], op=mybir.AluOpType.max)
                            nc.vector.tensor_add(sv_acc[mb][:], sv_acc[mb][:], svp[:])
                            nc.vector.tensor_add(sa_acc[mb][:], sa_acc[mb][:], sap[:])

                def s_emit(t):
                    tsl = bass.ts(t, 128)
                    s_sb = spool.tile([128, N], F32, tag="s_sb", name="s_sb")
                    for ch in range(NCH):
                        sl = bass.ts(ch, CHW)
                        ps = s_ps.tile([128, CHW], F32, tag="s_ps", name="s_ps")
                        for bi, (off, b) in enumerate(cblks):
                            nc.tensor.matmul(ps[:], lhsT=xT_blocks[bi][:, tsl],
                                             rhs=xT_blocks[bi][:, sl],
                                             start=(bi == 0), stop=False)
                        nc.tensor.matmul(ps[:], lhsT=ones128[:],
                                         rhs=sqm[:, sl], start=False, stop=True)
                        nc.scalar.copy(s_sb[:, sl], ps[:])
                    return s_sb

                def phase1(t, s_sb):
                    v24 = wpool.tile([128, 24], F32, tag="v24", name="v24")
                    i24 = wpool.tile([128, 24], U16, tag="i24", name="i24")
                    for r in range(3):
                        nc.vector.max(out=v24[:, 8 * r:8 * r + 8], in_=s_sb[:])
                        nc.vector.max_index(out=i24[:, 8 * r:8 * r + 8],
                                            in_max=v24[:, 8 * r:8 * r + 8],
                                            in_values=s_sb[:])
                        if r < 2:
                            nc.vector.match_replace(
                                out=s_sb[:],
                                in_to_replace=v24[:, 8 * r:8 * r + 8],
                                in_values=s_sb[:], imm_value=NEG)
                    nc.vector.tensor_copy(big_idx[:, t * KNN:(t + 1) * KNN],
                                          i24[:, 0:KNN])

                def fold(g0, g1):
                    # wrapped-16 layout I16[q, t*160 + 8r + d] (p = 16d+q), x8
                    csl = slice(g0 * 160, g1 * 160)
                    dstv = i16f[:].rearrange("p (tt r d) -> p tt r d",
                                             tt=NT, d=8)
                    srcv = big_idx[:].rearrange("p (tt r) -> p tt r", tt=NT)
                    for d in range(8):
                        nc.sync.dma_start(dstv[0:16, g0:g1, :, d],
                                          srcv[16 * d:16 * d + 16, g0:g1, :])
                    for k in (16, 32, 64):
                        nc.sync.dma_start(i16f[k:2 * k, csl], i16f[0:k, csl])

                def emit_gather(t, h):
                    hoff, hb = ohalves[h]
                    G = gpool.tile([128, KNN * hb], F32, tag="G", name="G")
                    g3 = G[:].rearrange("p (r c) -> p r c", r=KNN)
                    i16c = i16f[:].bitcast(I16)
                    Asl = A_dram[:, hoff:hoff + hb]
                    for (off, nr) in [(0, 8), (8, 8), (16, 4)]:
                        c0 = t * 160 + off * 8
                        nc.gpsimd.dma_gather(
                            out_ap=g3[:, off:off + nr, :], in_ap=Asl,
                            idxs_ap=i16c[:, c0:c0 + nr * 8],
                            num_idxs=nr * 128, num_idxs_reg=nr * 128,
                            elem_size=hb, elem_step=Co, queue_num=0)
                    return G

                def phase2(t, h, G):
                    tsl = bass.ts(t, 128)
                    hoff, hb = ohalves[h]
                    M = wpool.tile([128, 128], F32, tag="M", name="M")
                    nc.vector.tensor_reduce(
                        out=M[:, :hb],
                        in_=G[:].rearrange("p (r c) -> p c r", r=KNN),
                        axis=mybir.AxisListType.X,
                        op=mybir.AluOpType.max)
                    z = wpool.tile([128, 128], F32, tag="z", name="z")
                    nc.vector.tensor_add(z[:, :hb], M[:, :hb],
                                         Bc_all[:, t * Co + hoff:t * Co + hoff + hb])
                    tp = r_ps.tile([hb, 128], F32, tag="tr_ps", name="tr_ps")
                    nc.tensor.transpose(tp[:], z[:, 0:hb], ident[:])
                    v = wpool.tile([hb, 128], F32, tag="v_ep", name="v_ep")
                    nc.scalar.activation(v[:], tp[:],
                                         mybir.ActivationFunctionType.Identity,
                                         bias=bb_sb[l][h][:], scale=gs_sb[l][h][:])
                    w = wpool.tile([hb, 128], F32, tag="w_ep", name="w_ep")
                    nc.scalar.mul(w[:], v[:], 0.2)
                    nc.vector.tensor_tensor(out=nxt[h][:, tsl], in0=v[:], in1=w[:],
                                            op=mybir.AluOpType.max)

                # software-pipelined emission: gathers trail topk by a pair of
                # tiles; phase2 jobs trail their gathers by >= one tile
                from collections import deque
                jobs = deque()
                GT = 4 if NT % 4 == 0 else (2 if NT % 2 == 0 else 1)
                s_pend = deque()
                popped = 0
                h_ch = 0
                tpc = CHW // 128     # conv4 tiles per h-stage N-chunk
                for t in range(min(2, NT)):
                    s_pend.append(s_emit(t))
                for t in range(NT):
                    emit_tables(t)
                for g in range(NT // GT):
                    for t in range(g * GT, (g + 1) * GT):
                        if t + 2 < NT:
                            s_pend.append(s_emit(t + 2))
                        phase1(t, s_pend.popleft())
                        for _ in range(H):
                            if jobs:
                                jobs.popleft()()
                                popped += 1
                    fold(g * GT, (g + 1) * GT)
                    for t in range(g * GT, (g + 1) * GT):
                        for h in range(H):
                            G = emit_gather(t, h)
                            jobs.append(
                                lambda t=t, h=h, G=G: phase2(t, h, G))
                    if l == 3:
                        # conv4 tiles whose output columns are final
                        while (h_ch < NCH
                               and popped // H >= tpc * (h_ch + 1)):
                            emit_h_chunk(h_ch)
                            h_ch += 1
                while jobs:
                    jobs.popleft()()
                if l == 3:
                    while h_ch < NCH:
                        emit_h_chunk(h_ch)
                        h_ch += 1

                xT_blocks = nxt
                saved_xT.append(nxt)

            # =================== h pool finalization ===================
            gmax, gavg = [], []
            for mb in range(8):
                gm = wpool.tile([128, 1], F32, tag=f"gm{mb}", name=f"gm{mb}")
                wtmp = wpool.tile([128, 1], F32, tag="wtmp", name="wtmp")
                nc.scalar.mul(wtmp[:], hm_acc[mb][:], 0.2)
                nc.vector.tensor_tensor(gm[:], hm_acc[mb][:], wtmp[:],
                                        op=mybir.AluOpType.max)
                gmax.append(gm)
                ga = wpool.tile([128, 1], F32, tag=f"ga{mb}", name=f"ga{mb}")
                t1 = wpool.tile([128, 1], F32, tag="t1", name="t1")
                nc.scalar.mul(t1[:], sv_acc[mb][:], 0.6 / N)
                t2 = wpool.tile([128, 1], F32, tag="t2", name="t2")
                nc.scalar.mul(t2[:], sa_acc[mb][:], 0.4 / N)
                nc.vector.tensor_add(ga[:], t1[:], t2[:])
                gavg.append(ga)

            gvec = gmax + gavg

            # =================== head (row-vector form) ===================
            def row_affine_lrelu(ps_row, gsr, bbr, width, tag):
                """v = gs*z + b; out = max(v, 0.2v). All [1, width]."""
                v = wpool.tile([1, width], F32, tag=f"{tag}v")
                nc.vector.tensor_mul(v[:], ps_row[:], gsr[:])
                nc.vector.tensor_add(v[:], v[:], bbr[:])
                w = wpool.tile([1, width], F32, tag=f"{tag}w")
                nc.scalar.mul(w[:], v[:], 0.2)
                o = wpool.tile([1, width], F32, tag=f"{tag}o")
                nc.vector.tensor_tensor(o[:], v[:], w[:], op=mybir.AluOpType.max)
                return o

            ps1 = t_ps.tile([1, 512], F32, tag="vec_ps")
            for k in range(16):
                lt = strm.tile([128, 512], F32, tag="l1strm")
                nc.sync.dma_start(lt[:], l1_d[k * 128:(k + 1) * 128, :])
                nc.tensor.matmul(ps1[:], lhsT=gvec[k][:], rhs=lt[:],
                                 start=(k == 0), stop=(k == 15))
            z1r = row_affine_lrelu(ps1, gs6r, bb6r, 512, "z1")

            z1c = []
            for k in range(4):
                tp = r_ps.tile([128, 1], F32, tag="tr_ps")
                nc.tensor.transpose(tp[:], z1r[:, bass.ts(k, 128)], ident[0:1, 0:1])
                c = wpool.tile([128, 1], F32, tag=f"z1c{k}")
                nc.scalar.copy(c[:], tp[:])
                z1c.append(c)

            ps2 = t_ps.tile([1, 256], F32, tag="vec_ps")
            for k in range(4):
                nc.tensor.matmul(ps2[:], lhsT=z1c[k][:], rhs=l2_sb[k][:],
                                 start=(k == 0), stop=(k == 3))
            z2r = row_affine_lrelu(ps2, gs7r, bb7r, 256, "z2")

            z2c = []
            for k in range(2):
                tp = r_ps.tile([128, 1], F32, tag="tr_ps")
                nc.tensor.transpose(tp[:], z2r[:, bass.ts(k, 128)], ident[0:1, 0:1])
                c = wpool.tile([128, 1], F32, tag=f"z2c{k}")
                nc.scalar.copy(c[:], tp[:])
                z2c.append(c)

            ps3 = t_ps.tile([1, 10], F32, tag="vec_ps")
            for k in range(2):
                nc.tensor.matmul(ps3[:], lhsT=z2c[k][:], rhs=l3_sb[k][:],
                                 start=(k == 0), stop=(k == 1))
            osb = wpool.tile([1, 10], F32, tag="osb")
            nc.vector.tensor_add(osb[:], ps3[:], l3br[:])
            nc.sync.dma_start(out_d[:], osb[:])

    nc.compile()
    return nc


def make_in_maps(x, W1, g1, b1, W2, g2, b2, W3, g3, b3, W4, g4, b4,
                 W5, g5, b5, L1, g6, b6, L2, L2b, g7, b7, L3, L3b):
    f = np.float32
    Ws = [np.asarray(W1), np.asarray(W2), np.asarray(W3), np.asarray(W4)]
    gs = [np.asarray(g1), np.asarray(g2), np.asarray(g3), np.asarray(g4)]
    bs = [np.asarray(b1), np.asarray(b2), np.asarray(b3), np.asarray(b4)]
    x = np.asarray(x)
    common = {}
    for l in range(4):
        C = CS[l]
        Wn = Ws[l][:, :C]
        Wd = Ws[l][:, C:] - Wn
        common[f"WnT{l}"] = np.ascontiguousarray(Wn.T, dtype=f)
        common[f"WdT{l}"] = np.ascontiguousarray(Wd.T, dtype=f)
        common[f"gs{l}"] = np.ascontiguousarray((gs[l] * BN_SCALE).reshape(-1, 1), dtype=f)
        common[f"bb{l}"] = np.ascontiguousarray(bs[l].reshape(-1, 1), dtype=f)
    common["W5T"] = np.ascontiguousarray(np.asarray(W5).T, dtype=f)
    common["gs5"] = np.ascontiguousarray((np.asarray(g5) * BN_SCALE).reshape(-1, 1), dtype=f)
    common["bb5"] = np.ascontiguousarray(np.asarray(b5).reshape(-1, 1), dtype=f)
    common["L1T"] = np.ascontiguousarray(np.asarray(L1).T, dtype=f)
    common["gs6r"] = np.ascontiguousarray((np.asarray(g6) * BN_SCALE).reshape(1, -1), dtype=f)
    common["bb6r"] = np.ascontiguousarray(np.asarray(b6).reshape(1, -1), dtype=f)
    common["L2T"] = np.ascontiguousarray(np.asarray(L2).T, dtype=f)
    gs7v = (np.asarray(g7) * BN_SCALE).astype(f)
    common["gs7r"] = np.ascontiguousarray(gs7v.reshape(1, -1), dtype=f)
    common["bb7r"] = np.ascontiguousarray((gs7v * np.asarray(L2b) + np.asarray(b7)).reshape(1, -1), dtype=f)
    common["L3T"] = np.ascontiguousarray(np.asarray(L3).T, dtype=f)
    common["L3br"] = np.ascontiguousarray(np.asarray(L3b).reshape(1, -1), dtype=f)

    in_maps = []
    for i in range(x.shape[0]):
        m = dict(common)
        m["xT"] = np.ascontiguousarray(x[i].T, dtype=f)
        in_maps.append(m)
    return in_maps


_NC_CACHE = {}
_RUN_CACHE = {}


class _Runner:
    """Persistent executor for one compiled program.

    Same execution path as bass_utils.run_bass_kernel_spmd (bass2jax ->
    _bass_exec_p -> PJRT shard_map over 8 cores), but holds the jitted
    callable and keeps the weight operands resident on device across calls,
    so a call ships only xT (24 KB/core) instead of 7.2 MB/core. Weights are
    re-uploaded iff their packed bytes change (exact np.array_equal check).
    """

    def __init__(self, nc):
        import jax
        import concourse.mybir as _mybir
        from concourse import bass2jax
        from jax.sharding import Mesh, PartitionSpec
        from jax.experimental.shard_map import shard_map

        bass2jax.install_neuronx_cc_hook()
        self.nc = nc
        part_name = nc.partition_id_tensor.name if nc.partition_id_tensor else None
        in_names, out_names, out_avals, zero_shapes = [], [], [], []
        for alloc in nc.m.functions[0].allocations:
            if not isinstance(alloc, _mybir.MemoryLocationSet):
                continue
            name = alloc.memorylocations[0].name
            if alloc.kind == "ExternalInput":
                if name != part_name:
                    in_names.append(name)
            elif alloc.kind == "ExternalOutput":
                shape = tuple(alloc.tensor_shape)
                dtype = _mybir.dt.np(alloc.dtype)
                out_names.append(name)
                out_avals.append(jax.core.ShapedArray(shape, dtype))
                zero_shapes.append((shape, dtype))
        n_params = len(in_names)
        n_outs = len(out_avals)
        full_names = list(in_names) + out_names
        if part_name is not None:
            full_names.append(part_name)

        def _body(*args):
            operands = list(args)
            if part_name is not None:
                operands.append(bass2jax.partition_id_tensor())
            return tuple(bass2jax._bass_exec_p.bind(
                *operands,
                out_avals=tuple(out_avals),
                in_names=tuple(full_names),
                out_names=tuple(out_names),
                lowering_input_output_aliases=(),
                sim_require_finite=True,
                sim_require_nnan=True,
                nc=nc,
            ))

        devices = jax.devices()[:NCORES]
        assert len(devices) == NCORES
        mesh = Mesh(np.asarray(devices), ("core",))
        self.fn = jax.jit(
            shard_map(_body, mesh=mesh,
                      in_specs=(PartitionSpec("core"),) * (n_params + n_outs),
                      out_specs=(PartitionSpec("core"),) * n_outs,
                      check_rep=False),
            donate_argnums=tuple(range(n_params, n_params + n_outs)),
            keep_unused=True,
        )
        self.jax = jax
        self.sharding = jax.sharding.NamedSharding(mesh, PartitionSpec("core"))
        self.in_names = in_names
        self.out_avals = out_avals
        self.zero_shapes = zero_shapes
        self.host_w = None     # name -> packed np array (per-core shape)
        self.dev_w = None      # name -> committed jax array ([8*d0, ...])

    def set_weights(self, common):
        self.host_w = common
        self.dev_w = {
            k: self.jax.device_put(
                np.concatenate([v] * NCORES, axis=0), self.sharding)
            for k, v in common.items()
        }

    def dispatch(self, xT_cat):
        """Async-dispatch one execution against the cached device weights."""
        args = [xT_cat if name == "xT" else self.dev_w[name]
                for name in self.in_names]
        zeros = [np.zeros((NCORES * s[0], *s[1:]), dt)
                 for (s, dt) in self.zero_shapes]
        return self.fn(*args, *zeros)

    def __call__(self, xT_cat):
        return np.asarray(self.dispatch(xT_cat)[0])


_RAW_W_CACHE = {}   # N -> list of raw weight arrays from the last packing


def kernel(**inputs):
    x = np.asarray(inputs["x"])
    B, N, _ = x.shape
    assert B == NCORES
    if N not in _NC_CACHE:
        _NC_CACHE[N] = build(N=N)
    if N not in _RUN_CACHE:
        _RUN_CACHE[N] = _Runner(_NC_CACHE[N])
    run = _RUN_CACHE[N]

    xT_cat = np.ascontiguousarray(
        x.transpose(0, 2, 1).reshape(NCORES * CS[0], N), dtype=np.float32)

    # Speculatively dispatch against the cached device weights, then verify
    # the caller's weights still match while the RPC is in flight (~2.5 ms
    # bytes-compare off the critical path). On mismatch (rare: first call or
    # genuinely new weights) discard the speculative result, re-upload, redo.
    spec = run.dispatch(xT_cat) if run.dev_w is not None else None

    raww = [np.asarray(inputs[k]) for k in sorted(inputs) if k != "x"]
    prev = _RAW_W_CACHE.get(N)
    changed = prev is None or len(prev) != len(raww) or any(
        a.shape != b.shape or a.dtype != b.dtype or not np.array_equal(a, b)
        for a, b in zip(prev, raww))
    if changed:
        _RAW_W_CACHE[N] = raww
        in_maps = make_in_maps(**inputs)
        run.set_weights({k: v for k, v in in_maps[0].items() if k != "xT"})
        out = run(xT_cat)                  # redo with the fresh weights
    else:
        out = np.asarray(spec[0])          # speculation was valid
    return out.reshape(NCORES, -1).astype(np.float32)

